# revision 14
# baseline (speedup 1.0000x reference)
"""Trainium2 Bass kernel for nn_AMM_15126874817238 (dense_transformer).

Data-parallel over batch: 8 images -> 8 NeuronCores, one image per core.
Per-core plan (x: (128, 256, 256) f32):

Phase 1 - stream x in 16 chunks of 16 image rows; for each chunk:
  * q/k grouped 1x1 convs as block-diag 128x128 float32r matmuls (8 x N=512),
    fused 16x16 maxpool done while draining PSUM (DVE reduce / ACT copy+bf16 reduce)
  * v path algebraically fused: grouped1x1 + depthwise 4x4/s4 == one grouped
    4x4/s4 conv; computed as 16 tap-matmuls accumulating in PSUM with strided
    rhs APs straight from the x chunk (no im2col rearrangement)
  * shortcut 4x4/s4 conv computed FLIPPED (x patches stationary, weights moving)
    so its output lands (pixel, channel) -> LayerNorm over channels becomes a
    free-dim op; channel bias added via a K=1 ones-matmul into the same PSUM
Phase 2 - tiny pooled attention (4 heads, 128x128 score matrices), double
  softmax via exp/accum tricks, proj1 normal + exact GELU, proj2 flipped,
  both LayerNorms in (pixel, channel) layout, weighted add, DMA out (pix, oc).
Host - weight preprocessing (block-diag, tap splits, cpb bias table,
  transposed layouts) and final (pix,oc) -> (oc,H/4,W/4) unshard transpose.
"""
import sys
sys.path.insert(0, '/opt/trn_rl_repo')
from contextlib import ExitStack
import numpy as np

from concourse import bacc, mybir
from concourse import tile
from concourse.bass_utils import run_bass_kernel_spmd

f32 = mybir.dt.float32
f32r = mybir.dt.float32r
bf16 = mybir.dt.bfloat16
AX = mybir.AxisListType
OP = mybir.AluOpType
AF = mybir.ActivationFunctionType

B, C, H, W = 8, 128, 256, 256
OC, HEADS, NB, PS, OS = 256, 4, 4, 4, 16
GS = C // NB
HP, WP = H // PS, W // PS          # 64 x 64 pooled grid
NPIX = HP * WP                     # 4096
NCHUNK = 16                        # chunks of 16 image rows
ROWS = H // NCHUNK                 # 16
CPIX = (ROWS // PS) * WP           # 256 pooled pixels per chunk
NPT = NPIX // 128                  # 32 pixel tiles of 128

# How many of the 8 pool-drain subs per conv go through the ACT-copy route
# (True -> ACT copies PSUM->bf16 SBUF, DVE reduces at 4x; False -> DVE direct).
Q_ACT_ROUTE = [False] * 8
K_ACT_ROUTE = [True] * 7 + [False]


def _host_precompute(inp):
    fp = np.float32
    q_w = np.asarray(inp["q_w"], fp)[:, :, 0, 0]
    k_w = np.asarray(inp["k_w"], fp)[:, :, 0, 0]
    v_w = np.asarray(inp["v_w"], fp)[:, :, 0, 0]
    qp_w = np.asarray(inp["qp_w"], fp)[:, 0]
    kp_w = np.asarray(inp["kp_w"], fp)[:, 0]
    vp_w = np.asarray(inp["vp_w"], fp)[:, 0]

    def bd(wg):
        Wd = np.zeros((C, C), fp)
        for g in range(NB):
            Wd[g*GS:(g+1)*GS, g*GS:(g+1)*GS] = wg[g*GS:(g+1)*GS, :]
        return Wd
    Wq, Wk, Wv = bd(q_w), bd(k_w), bd(v_w)
    Wvf = Wv[:, :, None, None] * vp_w[:, None, :, :]      # (oc, ic, di, dj)

    qb_eff = np.asarray(inp["q_b"], fp) * qp_w.sum(axis=(1, 2)) + np.asarray(inp["qp_b"], fp)
    kb_eff = np.asarray(inp["k_b"], fp) * kp_w.sum(axis=(1, 2)) + np.asarray(inp["kp_b"], fp)
    vb_eff = np.asarray(inp["v_b"], fp) * vp_w.sum(axis=(1, 2)) + np.asarray(inp["vp_b"], fp)

    i = np.arange(C, dtype=fp)
    coords = i[None, :] - i[:, None]
    rel = coords / (C - 1) * 8.0
    rel = np.sign(rel) * np.log2(np.abs(rel) + 1.0) / np.log2(8.0)
    h1 = np.maximum(rel[..., None] @ np.asarray(inp["cpb_w1"], fp).T
                    + np.asarray(inp["cpb_b1"], fp), 0.0)
    battn = 1.0 / (1.0 + np.exp(-(h1 @ np.asarray(inp["cpb_w2"], fp).T)))   # (C, C, 4)
    battn = np.ascontiguousarray(battn.transpose(0, 2, 1))                   # (cq, h, d)

    scale = np.exp(np.minimum(np.asarray(inp["logit_scale"], fp), np.log(100.0))).reshape(HEADS)

    Wsc = np.asarray(inp["sc_w"], fp)                      # (256, 128, 4, 4)
    W1 = np.asarray(inp["proj_w1"], fp)[:, :, 0, 0]
    W2 = np.asarray(inp["proj_w2"], fp)[:, :, 0, 0]

    g1v = np.asarray(inp["norm_g"], fp)
    g2v = np.asarray(inp["sc_g"], fp)
    bev = np.asarray(inp["norm_be"], fp) + np.asarray(inp["sc_be"], fp)

    P = {}
    P["wq"] = np.ascontiguousarray(Wq.T)                   # (ic, oc) lhsT
    P["wk"] = np.ascontiguousarray(Wk.T)
    # (ic, tap, oc) lhsT per tap
    P["wv"] = np.ascontiguousarray(Wvf.transpose(1, 2, 3, 0).reshape(C, 16, C))
    # (ic, tap, oc) rhs per tap for flipped sc matmul
    import ml_dtypes
    P["wsc"] = np.ascontiguousarray(
        Wsc.transpose(1, 2, 3, 0).reshape(C, 16, OC)).astype(ml_dtypes.bfloat16)
    P["w1"] = np.ascontiguousarray(W1.T)                   # (ic, oc) lhsT
    P["w2"] = np.ascontiguousarray(W2.T)                   # (c1, oc) rhs (flipped)
    P["pb"] = np.stack([qb_eff, kb_eff, vb_eff, np.asarray(inp["proj_b1"], fp)], axis=1)  # (128,4)
    P["qpw"] = np.ascontiguousarray(qp_w.reshape(C, 9))
    P["kpw"] = np.ascontiguousarray(kp_w.reshape(C, 9))
    P["battn"] = battn                                     # (128, 4, 128)
    P["cbb2"] = np.stack([np.asarray(inp["sc_cb"], fp), np.asarray(inp["proj_b2"], fp)])[None]  # (1,2,256)
    P["ones1"] = np.ones((1, C), fp)
    P["ident"] = np.eye(C, dtype=fp)
    P["gbe"] = np.broadcast_to(np.stack([g1v, g2v, bev]), (C, 3, OC)).copy()  # (128,3,256)
    P["scale"] = scale
    return P


def _build(P, debug=False):
    nc = bacc.Bacc()

    x_d = nc.declare_dram_parameter("x", [C, H, W], f32r, isOutput=False)
    wq_d = nc.declare_dram_parameter("wq", [C, C], f32r, isOutput=False)
    wk_d = nc.declare_dram_parameter("wk", [C, C], f32r, isOutput=False)
    wv_d = nc.declare_dram_parameter("wv", [C, 16, C], f32r, isOutput=False)
    wsc_d = nc.declare_dram_parameter("wsc", [C, 16, OC], bf16, isOutput=False)
    w1_d = nc.declare_dram_parameter("w1", [C, C], f32r, isOutput=False)
    w2_d = nc.declare_dram_parameter("w2", [C, OC], f32r, isOutput=False)
    pb_d = nc.declare_dram_parameter("pb", [C, 4], f32, isOutput=False)
    qpw_d = nc.declare_dram_parameter("qpw", [C, 9], f32, isOutput=False)
    kpw_d = nc.declare_dram_parameter("kpw", [C, 9], f32, isOutput=False)
    battn_d = nc.declare_dram_parameter("battn", [C, HEADS, C], f32, isOutput=False)
    cbb2_d = nc.declare_dram_parameter("cbb2", [1, 2, OC], f32r, isOutput=False)
    ones1_d = nc.declare_dram_parameter("ones1", [1, C], f32r, isOutput=False)
    ident_d = nc.declare_dram_parameter("ident", [C, C], f32, isOutput=False)
    gbe_d = nc.declare_dram_parameter("gbe", [C, 3, OC], f32, isOutput=False)
    out_d = nc.declare_dram_parameter("out", [NPIX, OC], f32, isOutput=True)
    scale = P["scale"]

    dbg = {}
    if debug:
        def dbg_out(name, shape):
            dbg[name] = nc.declare_dram_parameter(name, list(shape), f32, isOutput=True)
        dbg_out("d_qpool", (C, 16, 16))
        dbg_out("d_kpool", (C, 16, 16))
        dbg_out("d_v", (C, NPIX))
        dbg_out("d_scT", (NPIX, OC))
        dbg_out("d_q196", (C, 196))
        dbg_out("d_p2", (C, HEADS, C))
        dbg_out("d_att", (C, NPIX))
        dbg_out("d_g1", (C, NPIX))
        dbg_out("d_o2T", (NPIX, OC))

    with tile.TileContext(nc) as tc, ExitStack() as es:
        cst = es.enter_context(tc.tile_pool(name="cst", bufs=1))
        big = es.enter_context(tc.tile_pool(name="big", bufs=1))
        xp = es.enter_context(tc.tile_pool(name="xp", bufs=2))
        wk_p = es.enter_context(tc.tile_pool(name="wkp", bufs=3))
        if True:
            # ---- constants ----
            wq_t = cst.tile([C, C], f32r)
            wk_t = cst.tile([C, C], f32r)
            wv_t = cst.tile([C, 16, C], f32r)
            wsc_t = cst.tile([C, 16, OC], bf16)
            w1_t = cst.tile([C, C], f32r)
            w2_t = cst.tile([C, OC], f32r)
            pb_t = cst.tile([C, 4], f32)
            qpw_t = cst.tile([C, 9], f32)
            kpw_t = cst.tile([C, 9], f32)
            battn_t = cst.tile([C, HEADS, C], f32)
            cbb2_t = cst.tile([1, 2, OC], f32r)
            ones1_t = cst.tile([1, C], f32r)
            ident_t = cst.tile([C, C], f32)
            gbe_t = cst.tile([C, 3, OC], f32)
            for t, d in [(wq_t, wq_d), (wk_t, wk_d), (wv_t, wv_d), (wsc_t, wsc_d),
                         (w1_t, w1_d), (w2_t, w2_d), (pb_t, pb_d), (qpw_t, qpw_d),
                         (kpw_t, kpw_d), (battn_t, battn_d), (cbb2_t, cbb2_d),
                         (ones1_t, ones1_d), (ident_t, ident_d), (gbe_t, gbe_d)]:
                nc.sync.dma_start(t[:], d[:])

            # ---- persistent accumulators ----
            qpool = big.tile([C, 16, 16], f32)
            kpool = big.tile([C, 16, 16], f32)
            vsb = big.tile([C, NPIX], f32r)        # v result (channel, pix)
            scT = big.tile([128, NPT, OC], bf16)   # shortcut (pix, oc), bf16
            sc_s1 = big.tile([128, NPT], f32)
            sc_ssq = big.tile([128, NPT], f32)

            # =============== PHASE 1: stream x ===============
            with ExitStack() as es1:
                psQ = es1.enter_context(tc.tile_pool(name="psQ", bufs=3, space="PSUM"))
                psV = es1.enter_context(tc.tile_pool(name="psV", bufs=2, space="PSUM"))
                psC = es1.enter_context(tc.tile_pool(name="psC", bufs=2, space="PSUM"))
                for ci in range(NCHUNK):
                    xc = xp.tile([C, ROWS, W], f32r, tag="xc")
                    nc.sync.dma_start(xc[:], x_d[:, ROWS*ci:ROWS*(ci+1), :])

                    # ---- q/k convs + fused 16x16 maxpool ----
                    for (wt, route, qtmp_tag, pool_t) in (
                            (wq_t, Q_ACT_ROUTE, "qtmp", qpool),
                            (wk_t, K_ACT_ROUTE, "ktmp", kpool)):
                        qtmp = wk_p.tile([C, 8, 16], f32, tag=qtmp_tag)
                        for s in range(8):
                            qps = psQ.tile([C, 512], f32, tag="qkps")
                            nc.tensor.matmul(qps[:], wt[:], xc[:, 2*s:2*s+2, :],
                                             start=True, stop=True)
                            if route[s]:
                                cp = wk_p.tile([C, 512], bf16, tag="cp16")
                                nc.scalar.copy(cp[:], qps[:])
                                src = cp
                            else:
                                src = qps
                            # (r=2, w=16, c=16) -> max over (r, c) per window
                            v_in = src[:].rearrange("p (r w c) -> p w r c", r=2, w=16, c=16)
                            nc.vector.tensor_reduce(
                                qtmp[:, s, :], v_in, axis=AX.XY, op=OP.max)
                        nc.vector.tensor_reduce(
                            pool_t[:, ci, :],
                            qtmp[:].rearrange("p s w -> p w s"),
                            axis=AX.X, op=OP.max)

                    # ---- fused v conv: 16 taps accumulate ----
                    vps = psV.tile([C, CPIX], f32, tag="vps")
                    for t in range(16):
                        di, dj = t // 4, t % 4
                        nc.tensor.matmul(vps[:], wv_t[:, t, :],
                                         xc[:, di:ROWS:4, dj:W:4],
                                         start=(t == 0), stop=(t == 15))
                    nc.scalar.activation(vsb[:, CPIX*ci:CPIX*(ci+1)],
                                         vps[:], AF.Identity,
                                         bias=pb_t[:, 2:3], scale=1.0)

                    # ---- tap-major staging (for sc stationary operand) ----
                    # T[c, di, dj, i, j] = xc[c, 4i+di, 4j+dj], bf16
                    tstg = xp.tile([C, 4, 4, 4, 64], bf16, tag="tstg")
                    xv = xc[:].rearrange("p (i di) (j dj) -> p di dj i j",
                                         i=4, di=4, j=64, dj=4)
                    nc.vector.tensor_copy(tstg[:, 0], xv[:, 0])
                    nc.vector.tensor_copy(tstg[:, 1], xv[:, 1])
                    nc.scalar.copy(tstg[:, 2], xv[:, 2])
                    nc.gpsimd.tensor_copy(tstg[:, 3], xv[:, 3])
                    tflat = tstg[:].rearrange("p di dj i j -> p (di dj) (i j)")

                    # ---- shortcut conv (flipped): 2 pixel-halves ----
                    for m in range(2):
                        pt = 2*ci + m                      # pixel-tile index
                        scps = psC.tile([128, OC], f32, tag="scps")
                        for t in range(16):
                            lhsT = tflat[:, t, 128*m:128*(m+1)]       # (128, 128) bf16
                            nc.tensor.matmul(scps[:], lhsT, wsc_t[:, t, :],
                                             start=(t == 0), stop=False)
                        nc.tensor.matmul(scps[:], ones1_t[:], cbb2_t[:, 0, :],
                                         start=False, stop=True)
                        # drain: copy (bf16) + per-pixel sum, then sumsq
                        nc.scalar.activation(scT[:, pt, :], scps[:], AF.Copy,
                                             accum_out=sc_s1[:, pt:pt+1])
                        sq = wk_p.tile([128, OC], f32, tag="scsq")
                        nc.scalar.activation(sq[:], scps[:], AF.Square,
                                             accum_out=sc_ssq[:, pt:pt+1])

            # =============== PHASE 2 ===============
            with ExitStack() as es2:
                p2 = es2.enter_context(tc.tile_pool(name="p2", bufs=1))
                sm = es2.enter_context(tc.tile_pool(name="sm", bufs=2))
                psS = es2.enter_context(tc.tile_pool(name="psS", bufs=2, space="PSUM"))
                psB = es2.enter_context(tc.tile_pool(name="psB", bufs=2, space="PSUM"))
                psO = es2.enter_context(tc.tile_pool(name="psO", bufs=2, space="PSUM"))
                if debug:
                    nc.sync.dma_start(dbg["d_qpool"][:], qpool[:])
                    nc.sync.dma_start(dbg["d_kpool"][:], kpool[:])
                    nc.sync.dma_start(dbg["d_v"][:], vsb[:].bitcast(f32))

                # ---- depthwise 3x3 on pooled maps + bias (gpsimd) ----
                q196 = p2.tile([C, 196], f32)
                k196 = p2.tile([C, 196], f32)
                for (pool_t, pw_t, bcol, acc) in ((qpool, qpw_t, 0, q196),
                                                  (kpool, kpw_t, 1, k196)):
                    accv = acc[:].rearrange("p (a b) -> p a b", a=14, b=14)
                    for t in range(9):
                        di, dj = t // 3, t % 3
                        src = pool_t[:, di:di+14, dj:dj+14]
                        if t == 0:
                            nc.vector.tensor_scalar(accv, src, pw_t[:, 0:1], None,
                                                    op0=OP.mult)
                        else:
                            nc.vector.scalar_tensor_tensor(
                                accv, src, pw_t[:, t:t+1], accv,
                                op0=OP.mult, op1=OP.add)
                    nc.vector.tensor_scalar(acc[:], acc[:], pb_t[:, bcol:bcol+1], None,
                                            op0=OP.add)
                if debug:
                    nc.sync.dma_start(dbg["d_q196"][:], q196[:])

                # ---- L2 norms per head (49 feats), fold logit scale into q ----
                qn = p2.tile([C, 196], f32)
                kn = p2.tile([C, 196], f32)
                for (src, dst, is_q) in ((q196, qn, True), (k196, kn, False)):
                    sq = sm.tile([C, 196], f32, tag="nsq")
                    nc.scalar.activation(sq[:], src[:], AF.Square)
                    ssq = sm.tile([C, HEADS], f32, tag="nssq")
                    nc.vector.tensor_reduce(
                        ssq[:], sq[:].rearrange("p (h f) -> p h f", h=4, f=49),
                        axis=AX.X, op=OP.add)
                    nrm = sm.tile([C, HEADS], f32, tag="nnrm")
                    nc.scalar.activation(nrm[:], ssq[:], AF.Sqrt)
                    nc.vector.tensor_scalar_max(nrm[:], nrm[:], 1e-12)
                    rcp = sm.tile([C, HEADS], f32, tag="nrcp")
                    nc.vector.reciprocal(rcp[:], nrm[:])
                    for h in range(HEADS):
                        s2 = float(scale[h]) if is_q else 1.0
                        nc.vector.tensor_scalar(
                            dst[:, 49*h:49*h+49], src[:, 49*h:49*h+49],
                            rcp[:, h:h+1], s2, op0=OP.mult, op1=OP.mult)

                # ---- transposes, sim, double softmax, att matmul ----
                lg = p2.tile([C, HEADS, C], f32)
                for h in range(HEADS):
                    tq = psS.tile([49, C], f32, tag="tps")
                    nc.tensor.transpose(tq[:], qn[:, 49*h:49*h+49], ident_t[:])
                    qnT = sm.tile([49, C], f32, tag="qnT")
                    nc.vector.tensor_copy(qnT[:], tq[:])
                    tk = psS.tile([49, C], f32, tag="tps")
                    nc.tensor.transpose(tk[:], kn[:, 49*h:49*h+49], ident_t[:])
                    knT = sm.tile([49, C], f32, tag="knT")
                    nc.vector.tensor_copy(knT[:], tk[:])
                    sps = psS.tile([C, C], f32, tag="sps")
                    nc.tensor.matmul(sps[:], qnT[:], knT[:], start=True, stop=True)
                    nc.vector.scalar_tensor_tensor(
                        lg[:, h, :], sps[:], 1.0, battn_t[:, h, :],
                        op0=OP.bypass, op1=OP.add)

                # softmax 1 over free dim
                stat = p2.tile([C, HEADS, 4], f32)   # [negmax, den1, min2, den2]
                nc.vector.tensor_reduce(stat[:, :, 0], lg[:], axis=AX.X,
                                        op=OP.max, negate=True)
                pr1 = p2.tile([C, HEADS, C], f32)
                for h in range(HEADS):
                    nc.scalar.activation(pr1[:, h, :], lg[:, h, :], AF.Exp,
                                         bias=stat[:, h, 0:1], scale=1.0,
                                         accum_out=stat[:, h, 1:2])
                rr = sm.tile([C, HEADS], f32, tag="rr")
                nc.vector.reciprocal(rr[:], stat[:, :, 1])
                for h in range(HEADS):
                    nc.vector.tensor_scalar_mul(pr1[:, h, :], pr1[:, h, :], rr[:, h:h+1])
                # softmax 2: softmax(1-p) via exp(min - p)
                nc.vector.tensor_reduce(stat[:, :, 2], pr1[:], axis=AX.X, op=OP.min)
                p2t = p2.tile([C, HEADS, C], f32)
                for h in range(HEADS):
                    nc.scalar.activation(p2t[:, h, :], pr1[:, h, :], AF.Exp,
                                         bias=stat[:, h, 2:3], scale=-1.0,
                                         accum_out=stat[:, h, 3:4])
                rr2 = sm.tile([C, HEADS], f32, tag="rr2")
                nc.vector.reciprocal(rr2[:], stat[:, :, 3])
                for h in range(HEADS):
                    nc.vector.tensor_scalar_mul(p2t[:, h, :], p2t[:, h, :], rr2[:, h:h+1])
                if debug:
                    nc.sync.dma_start(dbg["d_p2"][:], p2t[:])

                # att: out[cq, pix] = sum_d p2[cq, d] v[d, pix] ; lhsT = p2^T
                att = p2.tile([C, NPIX], f32r)
                for h in range(HEADS):
                    tp = psS.tile([C, C], f32, tag="tps")
                    nc.tensor.transpose(tp[:], p2t[:, h, :], ident_t[:])
                    simT = sm.tile([C, C], f32r, tag="simT")
                    nc.vector.tensor_copy(simT[:], tp[:])
                    for j in range(2):
                        aps = psB.tile([C, 512], f32, tag="bps")
                        nc.tensor.matmul(aps[:], simT[:],
                                         vsb[:, 1024*h+512*j:1024*h+512*(j+1)],
                                         start=True, stop=True)
                        nc.scalar.copy(att[:, 1024*h+512*j:1024*h+512*(j+1)],
                                       aps[:])
                if debug:
                    nc.sync.dma_start(dbg["d_att"][:], att[:].bitcast(f32))

                # ---- proj1 (normal) + gelu ----
                g1 = p2.tile([C, NPIX], f32r)
                for s in range(8):
                    pps = psB.tile([C, 512], f32, tag="bps")
                    nc.tensor.matmul(pps[:], w1_t[:], att[:, 512*s:512*(s+1)],
                                     start=True, stop=True)
                    nc.scalar.activation(g1[:, 512*s:512*(s+1)], pps[:],
                                         AF.Gelu, bias=pb_t[:, 3:4], scale=1.0)
                if debug:
                    nc.sync.dma_start(dbg["d_g1"][:], g1[:].bitcast(f32))

                # ---- proj2 (flipped) + LN stats ----
                o2T = big.tile([128, NPT, OC], bf16)
                o2_s1 = p2.tile([128, NPT], f32)
                o2_ssq = p2.tile([128, NPT], f32)
                for p in range(NPT):
                    ops_ = psO.tile([128, OC], f32, tag="o2ps")
                    nc.tensor.matmul(ops_[:], g1[:, 128*p:128*(p+1)], w2_t[:],
                                     start=True, stop=False)
                    nc.tensor.matmul(ops_[:], ones1_t[:], cbb2_t[:, 1, :],
                                     start=False, stop=True)
                    nc.scalar.activation(o2T[:, p, :], ops_[:], AF.Copy,
                                         accum_out=o2_s1[:, p:p+1])
                    sq2 = sm.tile([128, OC], f32, tag="o2sq")
                    nc.scalar.activation(sq2[:], ops_[:], AF.Square,
                                         accum_out=o2_ssq[:, p:p+1])
                if debug:
                    nc.gpsimd.dma_start(
                        dbg["d_scT"][:].rearrange("(t p) c -> p t c", p=128), scT[:])
                    nc.gpsimd.dma_start(
                        dbg["d_o2T"][:].rearrange("(t p) c -> p t c", p=128), o2T[:])

                # ---- batched LN stats math: rstd, -mu*rstd  (128, 32) ----
                def ln_stats(s1, ssq, tagp):
                    mu = sm.tile([128, NPT], f32, tag=tagp+"mu")
                    nc.vector.tensor_scalar_mul(mu[:], s1[:], 1.0 / OC)
                    var = sm.tile([128, NPT], f32, tag=tagp+"var")
                    # var + eps = (ssq/256 + 1e-5) - mu^2
                    nc.vector.tensor_scalar(var[:], ssq[:], 1.0 / OC, 1e-5,
                                            op0=OP.mult, op1=OP.add)
                    musq = sm.tile([128, NPT], f32, tag=tagp+"musq")
                    nc.vector.tensor_tensor(musq[:], mu[:], mu[:], op=OP.mult)
                    nc.vector.scalar_tensor_tensor(var[:], musq[:], -1.0, var[:],
                                                   op0=OP.mult, op1=OP.add)
                    std = sm.tile([128, NPT], f32, tag=tagp+"std")
                    nc.scalar.activation(std[:], var[:], AF.Sqrt)
                    rstd = p2.tile([128, NPT], f32, tag=tagp+"rstd")
                    nc.vector.reciprocal(rstd[:], std[:])
                    nmr = p2.tile([128, NPT], f32, tag=tagp+"nmr")
                    nc.vector.scalar_tensor_tensor(nmr[:], mu[:], -1.0, rstd[:],
                                                   op0=OP.mult, op1=OP.mult)
                    return rstd, nmr
                rstd_sc, nmr_sc = ln_stats(sc_s1, sc_ssq, "sc")
                rstd_o2, nmr_o2 = ln_stats(o2_s1, o2_ssq, "o2")

                # ---- LN apply + weighted add + out DMA ----
                for p in range(NPT):
                    u = sm.tile([128, OC], f32, tag="lnu")
                    nc.scalar.activation(u[:], o2T[:, p, :], AF.Identity,
                                         bias=nmr_o2[:, p:p+1], scale=rstd_o2[:, p:p+1])
                    w_ = sm.tile([128, OC], f32, tag="lnw")
                    nc.scalar.activation(w_[:], scT[:, p, :], AF.Identity,
                                         bias=nmr_sc[:, p:p+1], scale=rstd_sc[:, p:p+1])
                    t1 = sm.tile([128, OC], f32, tag="lnt1")
                    nc.vector.tensor_tensor(t1[:], u[:], gbe_t[:, 0, :], op=OP.mult)
                    t2 = sm.tile([128, OC], f32, tag="lnt2")
                    nc.gpsimd.tensor_tensor(t2[:], w_[:], gbe_t[:, 1, :], op=OP.mult)
                    fin = sm.tile([128, OC], f32, tag="fin")
                    nc.gpsimd.tensor_tensor(fin[:], t1[:], t2[:], op=OP.add)
                    nc.vector.tensor_tensor(fin[:], fin[:], gbe_t[:, 2, :], op=OP.add)
                    nc.sync.dma_start(out_d[128*p:128*(p+1), :], fin[:])

    nc.finalize()
    return nc


def kernel(**inputs):
    x = np.ascontiguousarray(np.asarray(inputs["x"], np.float32))
    P = _host_precompute(inputs)
    nc = _build(P)
    shared = {k: P[k] for k in ("wq", "wk", "wv", "wsc", "w1", "w2", "pb", "qpw",
                                "kpw", "battn", "cbb2", "ones1", "ident", "gbe")}
    in_maps = [dict(shared, x=np.ascontiguousarray(x[b])) for b in range(B)]
    res = run_bass_kernel_spmd(nc, in_maps, core_ids=list(range(B)))
    outs = []
    for b in range(B):
        oT = res.results[b]["out"]                 # (4096, 256)
        outs.append(oT.T.reshape(OC, HP, WP))
    out = np.stack(outs).astype(np.float32)
    return (out, x)


# revision 17
# speedup vs baseline: 1.1862x; 1.1862x over previous
"""Trainium2 Bass kernel for nn_AMM_15126874817238 (dense_transformer).

Data-parallel over batch: 8 images -> 8 NeuronCores, one image per core.
Per-core plan (x: (128, 256, 256) f32):

Phase 1 - stream x in 16 chunks of 16 image rows; for each chunk:
  * q/k grouped 1x1 convs as block-diag 128x128 float32r matmuls (8 x N=512),
    fused 16x16 maxpool done while draining PSUM (DVE reduce / ACT copy+bf16 reduce)
  * v path algebraically fused: grouped1x1 + depthwise 4x4/s4 == one grouped
    4x4/s4 conv; computed as 16 tap-matmuls accumulating in PSUM with strided
    rhs APs straight from the x chunk (no im2col rearrangement)
  * shortcut 4x4/s4 conv computed FLIPPED (x patches stationary, weights moving)
    so its output lands (pixel, channel) -> LayerNorm over channels becomes a
    free-dim op; channel bias added via a K=1 ones-matmul into the same PSUM
Phase 2 - tiny pooled attention (4 heads, 128x128 score matrices), double
  softmax via exp/accum tricks, proj1 normal + exact GELU, proj2 flipped,
  both LayerNorms in (pixel, channel) layout, weighted add, DMA out (pix, oc).
Host - weight preprocessing (block-diag, tap splits, cpb bias table,
  transposed layouts) and final (pix,oc) -> (oc,H/4,W/4) unshard transpose.
"""
import sys
sys.path.insert(0, '/opt/trn_rl_repo')
from contextlib import ExitStack
import numpy as np

from concourse import bacc, mybir
from concourse import tile
from concourse.bass_utils import run_bass_kernel_spmd

f32 = mybir.dt.float32
f32r = mybir.dt.float32r
bf16 = mybir.dt.bfloat16
AX = mybir.AxisListType
OP = mybir.AluOpType
AF = mybir.ActivationFunctionType

B, C, H, W = 8, 128, 256, 256
OC, HEADS, NB, PS, OS = 256, 4, 4, 4, 16
GS = C // NB
HP, WP = H // PS, W // PS          # 64 x 64 pooled grid
NPIX = HP * WP                     # 4096
NCHUNK = 16                        # chunks of 16 image rows
ROWS = H // NCHUNK                 # 16
CPIX = (ROWS // PS) * WP           # 256 pooled pixels per chunk
NPT = NPIX // 128                  # 32 pixel tiles of 128

# How many of the 8 pool-drain subs per conv go through the ACT-copy route
# (True -> ACT copies PSUM->bf16 SBUF, DVE reduces at 4x; False -> DVE direct).
Q_ACT_ROUTE = [False] * 8
K_ACT_ROUTE = [True] * 7 + [False]


def _host_precompute(inp):
    fp = np.float32
    q_w = np.asarray(inp["q_w"], fp)[:, :, 0, 0]
    k_w = np.asarray(inp["k_w"], fp)[:, :, 0, 0]
    v_w = np.asarray(inp["v_w"], fp)[:, :, 0, 0]
    qp_w = np.asarray(inp["qp_w"], fp)[:, 0]
    kp_w = np.asarray(inp["kp_w"], fp)[:, 0]
    vp_w = np.asarray(inp["vp_w"], fp)[:, 0]

    def bd(wg):
        Wd = np.zeros((C, C), fp)
        for g in range(NB):
            Wd[g*GS:(g+1)*GS, g*GS:(g+1)*GS] = wg[g*GS:(g+1)*GS, :]
        return Wd
    Wq, Wk, Wv = bd(q_w), bd(k_w), bd(v_w)
    Wvf = Wv[:, :, None, None] * vp_w[:, None, :, :]      # (oc, ic, di, dj)

    qb_eff = np.asarray(inp["q_b"], fp) * qp_w.sum(axis=(1, 2)) + np.asarray(inp["qp_b"], fp)
    kb_eff = np.asarray(inp["k_b"], fp) * kp_w.sum(axis=(1, 2)) + np.asarray(inp["kp_b"], fp)
    vb_eff = np.asarray(inp["v_b"], fp) * vp_w.sum(axis=(1, 2)) + np.asarray(inp["vp_b"], fp)

    i = np.arange(C, dtype=fp)
    coords = i[None, :] - i[:, None]
    rel = coords / (C - 1) * 8.0
    rel = np.sign(rel) * np.log2(np.abs(rel) + 1.0) / np.log2(8.0)
    h1 = np.maximum(rel[..., None] @ np.asarray(inp["cpb_w1"], fp).T
                    + np.asarray(inp["cpb_b1"], fp), 0.0)
    battn = 1.0 / (1.0 + np.exp(-(h1 @ np.asarray(inp["cpb_w2"], fp).T)))   # (C, C, 4)
    battn = np.ascontiguousarray(battn.transpose(0, 2, 1))                   # (cq, h, d)

    scale = np.exp(np.minimum(np.asarray(inp["logit_scale"], fp), np.log(100.0))).reshape(HEADS)

    Wsc = np.asarray(inp["sc_w"], fp)                      # (256, 128, 4, 4)
    W1 = np.asarray(inp["proj_w1"], fp)[:, :, 0, 0]
    W2 = np.asarray(inp["proj_w2"], fp)[:, :, 0, 0]

    g1v = np.asarray(inp["norm_g"], fp)
    g2v = np.asarray(inp["sc_g"], fp)
    bev = np.asarray(inp["norm_be"], fp) + np.asarray(inp["sc_be"], fp)

    import ml_dtypes
    b16 = ml_dtypes.bfloat16
    P = {}
    P["wq"] = np.ascontiguousarray(Wq.T).astype(b16)       # (ic, oc) lhsT
    P["wk"] = np.ascontiguousarray(Wk.T).astype(b16)
    # (ic, tap, oc) lhsT per tap
    P["wv"] = np.ascontiguousarray(
        Wvf.transpose(1, 2, 3, 0).reshape(C, 16, C)).astype(b16)
    # (ic, tap, oc) rhs per tap for flipped sc matmul
    P["wsc"] = np.ascontiguousarray(
        Wsc.transpose(1, 2, 3, 0).reshape(C, 16, OC)).astype(b16)
    P["w1"] = np.ascontiguousarray(W1.T).astype(b16)       # (ic, oc) lhsT
    P["w2"] = np.ascontiguousarray(W2.T).astype(b16)       # (c1, oc) rhs (flipped)
    P["pb"] = np.stack([qb_eff, kb_eff, vb_eff, np.asarray(inp["proj_b1"], fp)], axis=1)  # (128,4)
    P["qpw"] = np.ascontiguousarray(qp_w.reshape(C, 9))
    P["kpw"] = np.ascontiguousarray(kp_w.reshape(C, 9))
    P["battn"] = battn                                     # (128, 4, 128)
    P["cbb2"] = np.stack([np.asarray(inp["sc_cb"], fp),
                          np.asarray(inp["proj_b2"], fp)])[None].astype(b16)  # (1,2,256)
    P["ones1"] = np.ones((1, C), fp).astype(b16)
    P["ident"] = np.eye(C, dtype=fp)
    P["gbe"] = np.broadcast_to(np.stack([g1v, g2v, bev]), (C, 3, OC)).copy()  # (128,3,256)
    P["scale"] = scale
    return P


def _build(P, debug=False):
    nc = bacc.Bacc()

    x_d = nc.declare_dram_parameter("x", [C, H, W], f32, isOutput=False)
    wq_d = nc.declare_dram_parameter("wq", [C, C], bf16, isOutput=False)
    wk_d = nc.declare_dram_parameter("wk", [C, C], bf16, isOutput=False)
    wv_d = nc.declare_dram_parameter("wv", [C, 16, C], bf16, isOutput=False)
    wsc_d = nc.declare_dram_parameter("wsc", [C, 16, OC], bf16, isOutput=False)
    w1_d = nc.declare_dram_parameter("w1", [C, C], bf16, isOutput=False)
    w2_d = nc.declare_dram_parameter("w2", [C, OC], bf16, isOutput=False)
    pb_d = nc.declare_dram_parameter("pb", [C, 4], f32, isOutput=False)
    qpw_d = nc.declare_dram_parameter("qpw", [C, 9], f32, isOutput=False)
    kpw_d = nc.declare_dram_parameter("kpw", [C, 9], f32, isOutput=False)
    battn_d = nc.declare_dram_parameter("battn", [C, HEADS, C], f32, isOutput=False)
    cbb2_d = nc.declare_dram_parameter("cbb2", [1, 2, OC], bf16, isOutput=False)
    ones1_d = nc.declare_dram_parameter("ones1", [1, C], bf16, isOutput=False)
    ident_d = nc.declare_dram_parameter("ident", [C, C], f32, isOutput=False)
    gbe_d = nc.declare_dram_parameter("gbe", [C, 3, OC], f32, isOutput=False)
    out_d = nc.declare_dram_parameter("out", [NPIX, OC], f32, isOutput=True)
    scale = P["scale"]

    dbg = {}
    if debug:
        def dbg_out(name, shape):
            dbg[name] = nc.declare_dram_parameter(name, list(shape), f32, isOutput=True)
        dbg_out("d_qpool", (C, 16, 16))
        dbg_out("d_kpool", (C, 16, 16))
        dbg_out("d_v", (C, NPIX))
        dbg_out("d_scT", (NPIX, OC))
        dbg_out("d_q196", (C, 196))
        dbg_out("d_p2", (C, HEADS, C))
        dbg_out("d_att", (C, NPIX))
        dbg_out("d_g1", (C, NPIX))
        dbg_out("d_o2T", (NPIX, OC))

    with tile.TileContext(nc) as tc, ExitStack() as es:
        cst = es.enter_context(tc.tile_pool(name="cst", bufs=1))
        big = es.enter_context(tc.tile_pool(name="big", bufs=1))
        xp = es.enter_context(tc.tile_pool(name="xp", bufs=2))
        wk_p = es.enter_context(tc.tile_pool(name="wkp", bufs=3))
        if True:
            # ---- constants ----
            wq_t = cst.tile([C, C], bf16)
            wk_t = cst.tile([C, C], bf16)
            wv_t = cst.tile([C, 16, C], bf16)
            wsc_t = cst.tile([C, 16, OC], bf16)
            w1_t = cst.tile([C, C], bf16)
            w2_t = cst.tile([C, OC], bf16)
            pb_t = cst.tile([C, 4], f32)
            qpw_t = cst.tile([C, 9], f32)
            kpw_t = cst.tile([C, 9], f32)
            battn_t = cst.tile([C, HEADS, C], f32)
            cbb2_t = cst.tile([1, 2, OC], bf16)
            ones1_t = cst.tile([1, C], bf16)
            ident_t = cst.tile([C, C], f32)
            gbe_t = cst.tile([C, 3, OC], f32)
            for t, d in [(wq_t, wq_d), (wk_t, wk_d), (wv_t, wv_d), (wsc_t, wsc_d),
                         (w1_t, w1_d), (w2_t, w2_d), (pb_t, pb_d), (qpw_t, qpw_d),
                         (kpw_t, kpw_d), (battn_t, battn_d), (cbb2_t, cbb2_d),
                         (ones1_t, ones1_d), (ident_t, ident_d), (gbe_t, gbe_d)]:
                nc.sync.dma_start(t[:], d[:])

            # ---- persistent accumulators ----
            qpool = big.tile([C, 16, 16], f32)
            kpool = big.tile([C, 16, 16], f32)
            vsb = big.tile([C, NPIX], bf16)        # v result (channel, pix)
            scT = big.tile([128, NPT, OC], bf16)   # shortcut (pix, oc), bf16
            sc_s1 = big.tile([128, NPT], f32)
            sc_ssq = big.tile([128, NPT], f32)

            # =============== PHASE 1: stream x ===============
            with ExitStack() as es1:
                psQ = es1.enter_context(tc.tile_pool(name="psQ", bufs=3, space="PSUM"))
                psV = es1.enter_context(tc.tile_pool(name="psV", bufs=2, space="PSUM"))
                psC = es1.enter_context(tc.tile_pool(name="psC", bufs=2, space="PSUM"))
                for ci in range(NCHUNK):
                    xc = xp.tile([C, ROWS, W], f32, tag="xc")
                    nc.sync.dma_start(xc[:], x_d[:, ROWS*ci:ROWS*(ci+1), :])

                    # ---- tap-major bf16 staging (conversion + de-interleave) ----
                    # T[c, di, dj, i, j] = xc[c, 4i+di, 4j+dj], bf16
                    tstg = xp.tile([C, 4, 4, 4, 64], bf16, tag="tstg")
                    xv = xc[:].rearrange("p (i di) (j dj) -> p di dj i j",
                                         i=4, di=4, j=64, dj=4)
                    nc.vector.tensor_copy(tstg[:, 0], xv[:, 0])
                    nc.vector.tensor_copy(tstg[:, 1], xv[:, 1])
                    nc.scalar.copy(tstg[:, 2], xv[:, 2])
                    nc.gpsimd.tensor_copy(tstg[:, 3], xv[:, 3])
                    tflat = tstg[:].rearrange("p di dj i j -> p (di dj) (i j)")

                    # ---- q/k convs + fused 16x16 maxpool (tap-major order) ----
                    for (wt, route, qtmp_tag, pool_t) in (
                            (wq_t, Q_ACT_ROUTE, "qtmp", qpool),
                            (wk_t, K_ACT_ROUTE, "ktmp", kpool)):
                        qtmp = wk_p.tile([C, 8, 16], f32, tag=qtmp_tag)
                        for s in range(8):
                            qps = psQ.tile([C, 512], f32, tag="qkps")
                            nc.tensor.matmul(qps[:], wt[:], tflat[:, 2*s:2*s+2, :],
                                             start=True, stop=True)
                            if route[s]:
                                cp = wk_p.tile([C, 512], bf16, tag="cp16")
                                nc.scalar.copy(cp[:], qps[:])
                                src = cp
                            else:
                                src = qps
                            # psum flat = blk*64 + w*4 + j  (blk = tap-local x i)
                            v_in = src[:].rearrange("p (blk w j) -> p w blk j",
                                                    blk=8, w=16, j=4)
                            nc.vector.tensor_reduce(
                                qtmp[:, s, :], v_in, axis=AX.XY, op=OP.max)
                        nc.vector.tensor_reduce(
                            pool_t[:, ci, :],
                            qtmp[:].rearrange("p s w -> p w s"),
                            axis=AX.X, op=OP.max)

                    # ---- fused v conv: 16 taps accumulate ----
                    vps = psV.tile([C, CPIX], f32, tag="vps")
                    for t in range(16):
                        nc.tensor.matmul(vps[:], wv_t[:, t, :], tflat[:, t, :],
                                         start=(t == 0), stop=(t == 15))
                    nc.scalar.activation(vsb[:, CPIX*ci:CPIX*(ci+1)],
                                         vps[:], AF.Identity,
                                         bias=pb_t[:, 2:3], scale=1.0)

                    # ---- shortcut conv (flipped): 2 pixel-halves ----
                    for m in range(2):
                        pt = 2*ci + m                      # pixel-tile index
                        scps = psC.tile([128, OC], f32, tag="scps")
                        for t in range(16):
                            lhsT = tflat[:, t, 128*m:128*(m+1)]       # (128, 128) bf16
                            nc.tensor.matmul(scps[:], lhsT, wsc_t[:, t, :],
                                             start=(t == 0), stop=False)
                        nc.tensor.matmul(scps[:], ones1_t[:], cbb2_t[:, 0, :],
                                         start=False, stop=True)
                        # drain: copy (bf16) + per-pixel sum, then sumsq
                        nc.scalar.activation(scT[:, pt, :], scps[:], AF.Copy,
                                             accum_out=sc_s1[:, pt:pt+1])
                        sq = wk_p.tile([128, OC], f32, tag="scsq")
                        nc.scalar.activation(sq[:], scps[:], AF.Square,
                                             accum_out=sc_ssq[:, pt:pt+1])

            # =============== PHASE 2 ===============
            with ExitStack() as es2:
                p2 = es2.enter_context(tc.tile_pool(name="p2", bufs=1))
                sm = es2.enter_context(tc.tile_pool(name="sm", bufs=2))
                psS = es2.enter_context(tc.tile_pool(name="psS", bufs=2, space="PSUM"))
                psB = es2.enter_context(tc.tile_pool(name="psB", bufs=2, space="PSUM"))
                psO = es2.enter_context(tc.tile_pool(name="psO", bufs=2, space="PSUM"))
                if debug:
                    nc.sync.dma_start(dbg["d_qpool"][:], qpool[:])
                    nc.sync.dma_start(dbg["d_kpool"][:], kpool[:])
                    nc.gpsimd.dma_start(dbg["d_v"][:], vsb[:])

                # ---- depthwise 3x3 on pooled maps + bias (gpsimd) ----
                q196 = p2.tile([C, 196], f32)
                k196 = p2.tile([C, 196], f32)
                for (pool_t, pw_t, bcol, acc) in ((qpool, qpw_t, 0, q196),
                                                  (kpool, kpw_t, 1, k196)):
                    accv = acc[:].rearrange("p (a b) -> p a b", a=14, b=14)
                    for t in range(9):
                        di, dj = t // 3, t % 3
                        src = pool_t[:, di:di+14, dj:dj+14]
                        if t == 0:
                            nc.vector.tensor_scalar(accv, src, pw_t[:, 0:1], None,
                                                    op0=OP.mult)
                        else:
                            nc.vector.scalar_tensor_tensor(
                                accv, src, pw_t[:, t:t+1], accv,
                                op0=OP.mult, op1=OP.add)
                    nc.vector.tensor_scalar(acc[:], acc[:], pb_t[:, bcol:bcol+1], None,
                                            op0=OP.add)
                if debug:
                    nc.sync.dma_start(dbg["d_q196"][:], q196[:])

                # ---- L2 norms per head (49 feats), fold logit scale into q ----
                qn = p2.tile([C, 196], f32)
                kn = p2.tile([C, 196], f32)
                for (src, dst, is_q) in ((q196, qn, True), (k196, kn, False)):
                    sq = sm.tile([C, 196], f32, tag="nsq")
                    nc.scalar.activation(sq[:], src[:], AF.Square)
                    ssq = sm.tile([C, HEADS], f32, tag="nssq")
                    nc.vector.tensor_reduce(
                        ssq[:], sq[:].rearrange("p (h f) -> p h f", h=4, f=49),
                        axis=AX.X, op=OP.add)
                    nrm = sm.tile([C, HEADS], f32, tag="nnrm")
                    nc.scalar.activation(nrm[:], ssq[:], AF.Sqrt)
                    nc.vector.tensor_scalar_max(nrm[:], nrm[:], 1e-12)
                    rcp = sm.tile([C, HEADS], f32, tag="nrcp")
                    nc.vector.reciprocal(rcp[:], nrm[:])
                    for h in range(HEADS):
                        s2 = float(scale[h]) if is_q else 1.0
                        nc.vector.tensor_scalar(
                            dst[:, 49*h:49*h+49], src[:, 49*h:49*h+49],
                            rcp[:, h:h+1], s2, op0=OP.mult, op1=OP.mult)

                # ---- transposes, sim, double softmax, att matmul ----
                lg = p2.tile([C, HEADS, C], f32)
                for h in range(HEADS):
                    tq = psS.tile([49, C], f32, tag="tps")
                    nc.tensor.transpose(tq[:], qn[:, 49*h:49*h+49], ident_t[:])
                    qnT = sm.tile([49, C], f32, tag="qnT")
                    nc.vector.tensor_copy(qnT[:], tq[:])
                    tk = psS.tile([49, C], f32, tag="tps")
                    nc.tensor.transpose(tk[:], kn[:, 49*h:49*h+49], ident_t[:])
                    knT = sm.tile([49, C], f32, tag="knT")
                    nc.vector.tensor_copy(knT[:], tk[:])
                    sps = psS.tile([C, C], f32, tag="sps")
                    nc.tensor.matmul(sps[:], qnT[:], knT[:], start=True, stop=True)
                    nc.vector.scalar_tensor_tensor(
                        lg[:, h, :], sps[:], 1.0, battn_t[:, h, :],
                        op0=OP.bypass, op1=OP.add)

                # softmax 1 over free dim
                stat = p2.tile([C, HEADS, 4], f32)   # [negmax, den1, min2, den2]
                nc.vector.tensor_reduce(stat[:, :, 0], lg[:], axis=AX.X,
                                        op=OP.max, negate=True)
                pr1 = p2.tile([C, HEADS, C], f32)
                for h in range(HEADS):
                    nc.scalar.activation(pr1[:, h, :], lg[:, h, :], AF.Exp,
                                         bias=stat[:, h, 0:1], scale=1.0,
                                         accum_out=stat[:, h, 1:2])
                rr = sm.tile([C, HEADS], f32, tag="rr")
                nc.vector.reciprocal(rr[:], stat[:, :, 1])
                for h in range(HEADS):
                    nc.vector.tensor_scalar_mul(pr1[:, h, :], pr1[:, h, :], rr[:, h:h+1])
                # softmax 2: softmax(1-p) via exp(min - p)
                nc.vector.tensor_reduce(stat[:, :, 2], pr1[:], axis=AX.X, op=OP.min)
                p2t = p2.tile([C, HEADS, C], f32)
                for h in range(HEADS):
                    nc.scalar.activation(p2t[:, h, :], pr1[:, h, :], AF.Exp,
                                         bias=stat[:, h, 2:3], scale=-1.0,
                                         accum_out=stat[:, h, 3:4])
                rr2 = sm.tile([C, HEADS], f32, tag="rr2")
                nc.vector.reciprocal(rr2[:], stat[:, :, 3])
                for h in range(HEADS):
                    nc.vector.tensor_scalar_mul(p2t[:, h, :], p2t[:, h, :], rr2[:, h:h+1])
                if debug:
                    nc.sync.dma_start(dbg["d_p2"][:], p2t[:])

                # att: out[cq, pix] = sum_d p2[cq, d] v[d, pix] ; lhsT = p2^T
                att = p2.tile([C, NPIX], bf16)
                for h in range(HEADS):
                    tp = psS.tile([C, C], f32, tag="tps")
                    nc.tensor.transpose(tp[:], p2t[:, h, :], ident_t[:])
                    simT = sm.tile([C, C], bf16, tag="simT")
                    nc.vector.tensor_copy(simT[:], tp[:])
                    for j in range(2):
                        aps = psB.tile([C, 512], f32, tag="bps")
                        nc.tensor.matmul(aps[:], simT[:],
                                         vsb[:, 1024*h+512*j:1024*h+512*(j+1)],
                                         start=True, stop=True)
                        nc.scalar.copy(att[:, 1024*h+512*j:1024*h+512*(j+1)],
                                       aps[:])
                if debug:
                    nc.gpsimd.dma_start(dbg["d_att"][:], att[:])

                # ---- proj1 (normal) + gelu ----
                g1 = p2.tile([C, NPIX], bf16)
                for s in range(8):
                    pps = psB.tile([C, 512], f32, tag="bps")
                    nc.tensor.matmul(pps[:], w1_t[:], att[:, 512*s:512*(s+1)],
                                     start=True, stop=True)
                    nc.scalar.activation(g1[:, 512*s:512*(s+1)], pps[:],
                                         AF.Gelu, bias=pb_t[:, 3:4], scale=1.0)
                if debug:
                    nc.gpsimd.dma_start(dbg["d_g1"][:], g1[:])

                # ---- proj2 (flipped) + LN stats ----
                o2T = big.tile([128, NPT, OC], bf16)
                o2_s1 = p2.tile([128, NPT], f32)
                o2_ssq = p2.tile([128, NPT], f32)
                for p in range(NPT):
                    ops_ = psO.tile([128, OC], f32, tag="o2ps")
                    nc.tensor.matmul(ops_[:], g1[:, 128*p:128*(p+1)], w2_t[:],
                                     start=True, stop=False)
                    nc.tensor.matmul(ops_[:], ones1_t[:], cbb2_t[:, 1, :],
                                     start=False, stop=True)
                    nc.scalar.activation(o2T[:, p, :], ops_[:], AF.Copy,
                                         accum_out=o2_s1[:, p:p+1])
                    sq2 = sm.tile([128, OC], f32, tag="o2sq")
                    nc.scalar.activation(sq2[:], ops_[:], AF.Square,
                                         accum_out=o2_ssq[:, p:p+1])
                if debug:
                    nc.gpsimd.dma_start(
                        dbg["d_scT"][:].rearrange("(t p) c -> p t c", p=128), scT[:])
                    nc.gpsimd.dma_start(
                        dbg["d_o2T"][:].rearrange("(t p) c -> p t c", p=128), o2T[:])

                # ---- batched LN stats math: rstd, -mu*rstd  (128, 32) ----
                def ln_stats(s1, ssq, tagp):
                    mu = sm.tile([128, NPT], f32, tag=tagp+"mu")
                    nc.vector.tensor_scalar_mul(mu[:], s1[:], 1.0 / OC)
                    var = sm.tile([128, NPT], f32, tag=tagp+"var")
                    # var + eps = (ssq/256 + 1e-5) - mu^2
                    nc.vector.tensor_scalar(var[:], ssq[:], 1.0 / OC, 1e-5,
                                            op0=OP.mult, op1=OP.add)
                    musq = sm.tile([128, NPT], f32, tag=tagp+"musq")
                    nc.vector.tensor_tensor(musq[:], mu[:], mu[:], op=OP.mult)
                    nc.vector.scalar_tensor_tensor(var[:], musq[:], -1.0, var[:],
                                                   op0=OP.mult, op1=OP.add)
                    std = sm.tile([128, NPT], f32, tag=tagp+"std")
                    nc.scalar.activation(std[:], var[:], AF.Sqrt)
                    rstd = p2.tile([128, NPT], f32, tag=tagp+"rstd")
                    nc.vector.reciprocal(rstd[:], std[:])
                    nmr = p2.tile([128, NPT], f32, tag=tagp+"nmr")
                    nc.vector.scalar_tensor_tensor(nmr[:], mu[:], -1.0, rstd[:],
                                                   op0=OP.mult, op1=OP.mult)
                    return rstd, nmr
                rstd_sc, nmr_sc = ln_stats(sc_s1, sc_ssq, "sc")
                rstd_o2, nmr_o2 = ln_stats(o2_s1, o2_ssq, "o2")

                # ---- LN apply + weighted add + out DMA ----
                for p in range(NPT):
                    u = sm.tile([128, OC], f32, tag="lnu")
                    nc.scalar.activation(u[:], o2T[:, p, :], AF.Identity,
                                         bias=nmr_o2[:, p:p+1], scale=rstd_o2[:, p:p+1])
                    w_ = sm.tile([128, OC], f32, tag="lnw")
                    nc.scalar.activation(w_[:], scT[:, p, :], AF.Identity,
                                         bias=nmr_sc[:, p:p+1], scale=rstd_sc[:, p:p+1])
                    t1 = sm.tile([128, OC], f32, tag="lnt1")
                    nc.vector.tensor_tensor(t1[:], u[:], gbe_t[:, 0, :], op=OP.mult)
                    t2 = sm.tile([128, OC], f32, tag="lnt2")
                    nc.gpsimd.tensor_tensor(t2[:], w_[:], gbe_t[:, 1, :], op=OP.mult)
                    fin = sm.tile([128, OC], f32, tag="fin")
                    nc.gpsimd.tensor_tensor(fin[:], t1[:], t2[:], op=OP.add)
                    nc.vector.tensor_tensor(fin[:], fin[:], gbe_t[:, 2, :], op=OP.add)
                    nc.sync.dma_start(out_d[128*p:128*(p+1), :], fin[:])

    nc.finalize()
    return nc


def kernel(**inputs):
    x = np.ascontiguousarray(np.asarray(inputs["x"], np.float32))
    P = _host_precompute(inputs)
    nc = _build(P)
    shared = {k: P[k] for k in ("wq", "wk", "wv", "wsc", "w1", "w2", "pb", "qpw",
                                "kpw", "battn", "cbb2", "ones1", "ident", "gbe")}
    in_maps = [dict(shared, x=np.ascontiguousarray(x[b])) for b in range(B)]
    res = run_bass_kernel_spmd(nc, in_maps, core_ids=list(range(B)))
    outs = []
    for b in range(B):
        oT = res.results[b]["out"]                 # (4096, 256)
        outs.append(oT.T.reshape(OC, HP, WP))
    out = np.stack(outs).astype(np.float32)
    return (out, x)
